# revision 10
# baseline (speedup 1.0000x reference)
"""Trainium2 Bass kernel for nn_Interactor (attention-augmented LSTM).

Problem: B=64, T=512, L=48, DV=DS=H=512.
  per step t: proj_V = x_t W_V^T; proj_R = h W_R^T
              e = tanh(proj_R[:,None,:] + proj_S + proj_V[:,None,:])
              alpha = softmax(e @ w, axis=L); h_ts = alpha @ h_s
              gates = [x_t, h_ts] W_ih^T + h W_hh^T + b; LSTM cell update.

Sharding: data-parallel over batch: 8 cores x 8 batch rows, weights replicated.
Within a core the 8 rows are processed as TWO independent groups of 4 that are
software-pipelined against each other to hide the serial chain latency.

Design highlights (cost-model driven):
 - All matmul MOVING operands are bf16 (4x cheaper than f32 per column);
   the recurrent state is carried as bf16 Hc = 2h.
 - e = tanh(PS + bcast(rv)) is built by PE "injection": ident@PS_bf16 plus a
   stride-0-broadcast matmul of rv (PSUM accumulation), so the only vector-
   engine work on the e path is the Act tanh reading PSUM directly.
 - softmax: beta stays in [L, b] layout; Act exp -> GpSimd partition_all_reduce
   (denominator on all 48 partitions) -> DVE reciprocal + multiply -> alpha.
 - Z-trick: Z_b = W_ihS @ h_s[b]^T is precomputed on device ([48, b, 4H] bf16);
   the attention result enters the gate pre-activations directly as per-column
   matmuls Z_b @ alpha_b accumulated into the SAME PSUM bank as
   ident@GV + W_hh@Hc (no h_ts materialization / normalization / reinjection).
 - Sigmoids via tanh with host-prescaled weights (i/f/o rows x0.5); the cell
   update is 3 fused DVE scalar_tensor_tensor stages; output is written as
   bf16 Hc=2h and halved on the host.
 - DRAM layouts keep per-partition runs >= 256B so DMA descriptor counts stay
   small (the [t,m,p,b] layouts of the old design cost ~0.5ms of hidden DMA).
"""

import numpy as np

import concourse.bass as bass
import concourse.mybir as mybir
import concourse.bass_isa as bass_isa
import concourse.tile as tile
from concourse import bacc
from concourse.bass_utils import run_bass_kernel_spmd

F32 = mybir.dt.float32
BF16 = mybir.dt.bfloat16
AF = mybir.ActivationFunctionType
ALU = mybir.AluOpType

B, T_FULL, L = 64, 512, 48
DV, DS, H = 512, 512, 512
G4 = 4 * H
NCORES = 8
BLOC = B // NCORES   # 8 batch rows per core
NG = 2               # pipelined groups per core
BG = BLOC // NG      # 4 batch rows per group
KH = H // 128        # 4
KM = G4 // 128       # 16
BL = BLOC * L        # 384
GL = BG * L          # 192
HB = 8               # h ring steps per output DMA


def build_nc(T=T_FULL, Tc=32):
    assert T % Tc == 0 and T % HB == 0
    nc = bacc.Bacc()

    # ---- DRAM I/O ----
    hvT = nc.declare_dram_parameter("hvT", [DV, T * BLOC], BF16, isOutput=False)
    hsT = nc.declare_dram_parameter("hsT", [DS, BL], BF16, isOutput=False)
    WS_T = nc.declare_dram_parameter("WS_T", [DS, H], BF16, isOutput=False)
    WV_T = nc.declare_dram_parameter("WV_T", [DV, H], BF16, isOutput=False)
    WihV_T = nc.declare_dram_parameter("WihV_T", [DV, G4], BF16, isOutput=False)
    WihS_T = nc.declare_dram_parameter("WihS_T", [DS, G4], BF16, isOutput=False)
    Whh_T = nc.declare_dram_parameter("Whh_T", [H, G4], BF16, isOutput=False)
    WR_T = nc.declare_dram_parameter("WR_T", [H, H], BF16, isOutput=False)
    wvec = nc.declare_dram_parameter("wvec", [128, KH], BF16, isOutput=False)
    biasRSV = nc.declare_dram_parameter("biasRSV", [128, KH], F32, isOutput=False)
    biasIH = nc.declare_dram_parameter("biasIH", [128, KM], F32, isOutput=False)
    bw128 = nc.declare_dram_parameter("bw128", [128, 1], F32, isOutput=False)
    identb = nc.declare_dram_parameter("identb", [128, 128], BF16, isOutput=False)
    out_c = nc.declare_dram_parameter("out_c", [NG, KH, 128, T, BG], BF16,
                                      isOutput=True)

    # ---- internal DRAM for precomputed projections ----
    GV_d = nc.dram_tensor("GV_d", [KM, 128, T, BLOC], BF16)
    PV_d = nc.dram_tensor("PV_d", [KH, 128, T, BLOC], BF16)

    NT = T * BLOC
    NCW = min(512, NT)
    n_nc = NT // NCW

    with tile.TileContext(nc) as tc:
        with (
            tc.tile_pool(name="res", bufs=1) as res,
            tc.tile_pool(name="stream", bufs=2) as stream,
            tc.tile_pool(name="houtA", bufs=2) as houtA,
            tc.tile_pool(name="houtB", bufs=2) as houtB,
            tc.tile_pool(name="stA", bufs=2) as stA,
            tc.tile_pool(name="stB", bufs=2) as stB,
            tc.tile_pool(name="wkA", bufs=2) as wkA,
            tc.tile_pool(name="wkB", bufs=2) as wkB,
        ):
            hout = (houtA, houtB)
            st = (stA, stB)
            wk = (wkA, wkB)

            # ---------- resident loads ----------
            wr_sb = res.tile([128, KH, H], BF16, tag="wr")
            nc.sync.dma_start(out=wr_sb, in_=WR_T.rearrange("(k p) m -> p k m", p=128))
            whh_sb = res.tile([128, KH, G4], BF16, tag="whh")
            nc.sync.dma_start(out=whh_sb, in_=Whh_T.rearrange("(k p) m -> p k m", p=128))
            wvec_sb = res.tile([128, KH], BF16, tag="wvec")
            nc.sync.dma_start(out=wvec_sb, in_=wvec[:, :])
            bw_sb = res.tile([128, 1], F32, tag="bw")
            nc.sync.dma_start(out=bw_sb, in_=bw128[:, :])
            identb_sb = res.tile([128, 128], BF16, tag="identb")
            nc.sync.dma_start(out=identb_sb, in_=identb[:, :])
            ps_bf = res.tile([128, KH, BL], BF16, tag="psbf")
            z_sb = res.tile([48, BLOC, G4], BF16, tag="z")
            hzero = res.tile([128, KH, BG], BF16, tag="h0")
            nc.vector.memset(hzero, 0.0)
            czero = res.tile([128, KH, BG], F32, tag="c0")
            nc.vector.memset(czero, 0.0)

            # ---------- precompute phase ----------
            with (
                tc.tile_pool(name="prew", bufs=1) as prew,
                tc.tile_pool(name="prehv", bufs=4) as prehv,
                tc.tile_pool(name="prestg", bufs=2) as prestg,
                tc.tile_pool(name="prepsum", bufs=4, space="PSUM") as prepsum,
            ):
                hs_sb = prew.tile([128, KH, BL], BF16, tag="hs")
                nc.sync.dma_start(out=hs_sb, in_=hsT.rearrange("(k p) n -> p k n", p=128))
                brsv_sb = prew.tile([128, KH], F32, tag="brsv")
                nc.sync.dma_start(out=brsv_sb, in_=biasRSV[:, :])
                bih_sb = prew.tile([128, KM], F32, tag="bih")
                nc.sync.dma_start(out=bih_sb, in_=biasIH[:, :])
                ws_sb = prew.tile([128, KH, H], BF16, tag="ws")
                nc.sync.dma_start(out=ws_sb, in_=WS_T.rearrange("(k p) m -> p k m", p=128))
                wv_sb = prew.tile([128, KH, H], BF16, tag="wv")
                nc.sync.dma_start(out=wv_sb, in_=WV_T.rearrange("(k p) m -> p k m", p=128))
                wihv_sb = prew.tile([128, KH, G4], BF16, tag="wihv")
                nc.sync.dma_start(out=wihv_sb, in_=WihV_T.rearrange("(k p) m -> p k m", p=128))
                wihs_sb = prew.tile([128, KH, G4], BF16, tag="wihs")
                nc.sync.dma_start(out=wihs_sb, in_=WihS_T.rearrange("(k p) m -> p k m", p=128))

                # PS = W_S @ hsT + biasRSV -> bf16
                for m in range(KH):
                    pps = prepsum.tile([128, 512], F32, tag="pp")
                    for kc in range(KH):
                        nc.tensor.matmul(
                            pps[:, :BL],
                            ws_sb[:, kc, m * 128:(m + 1) * 128],
                            hs_sb[:, kc, :],
                            start=(kc == 0), stop=(kc == KH - 1))
                    nc.vector.tensor_scalar_add(
                        ps_bf[:, m, :], pps[:, :BL], brsv_sb[:, m:m + 1])

                # Z_b = W_ihS @ h_s[b]^T in [48, b, 4H] bf16 layout
                for b in range(BLOC):
                    for n4 in range(4):
                        pz = prepsum.tile([48, 512], F32, tag="pz")
                        for kc in range(KH):
                            nc.tensor.matmul(
                                pz,
                                hs_sb[:, kc, b * L:(b + 1) * L],
                                wihs_sb[:, kc, n4 * 512:(n4 + 1) * 512],
                                start=(kc == 0), stop=(kc == KH - 1))
                        dst = z_sb[:, b, n4 * 512:(n4 + 1) * 512]
                        if (b * 4 + n4) % 2 == 0:
                            nc.scalar.copy(dst, pz)
                        else:
                            nc.vector.tensor_copy(dst, pz)

                # PV / GV over hvT N-chunks
                for ncnk in range(n_nc):
                    nsl = slice(ncnk * NCW, (ncnk + 1) * NCW)
                    t0 = ncnk * NCW // BLOC
                    tw = NCW // BLOC
                    hv_t = []
                    for kc in range(KH):
                        t_ = prehv.tile([128, NCW], BF16, tag="hv")
                        nc.sync.dma_start(
                            out=t_, in_=hvT[kc * 128:(kc + 1) * 128, nsl])
                        hv_t.append(t_)
                    for m in range(KH):
                        ppv = prepsum.tile([128, NCW], F32, tag="pp")
                        for kc in range(KH):
                            nc.tensor.matmul(
                                ppv, wv_sb[:, kc, m * 128:(m + 1) * 128],
                                hv_t[kc], start=(kc == 0), stop=(kc == KH - 1))
                        stg = prestg.tile([128, NCW], BF16, tag="pvstg")
                        nc.scalar.copy(stg, ppv)
                        nc.sync.dma_start(
                            out=PV_d[m, :, t0:t0 + tw, :],
                            in_=stg.rearrange("p (t b) -> p t b", b=BLOC))
                    for m in range(KM):
                        pgv = prepsum.tile([128, NCW], F32, tag="pp")
                        for kc in range(KH):
                            nc.tensor.matmul(
                                pgv, wihv_sb[:, kc, m * 128:(m + 1) * 128],
                                hv_t[kc], start=(kc == 0), stop=(kc == KH - 1))
                        stg = prestg.tile([128, NCW], BF16, tag="gvstg")
                        nc.vector.tensor_scalar_add(stg, pgv, bih_sb[:, m:m + 1])
                        nc.sync.dma_start(
                            out=GV_d[m, :, t0:t0 + tw, :],
                            in_=stg.rearrange("p (t b) -> p t b", b=BLOC))

            # ---------- recurrence ----------
            psE = [None, None]
            psG = [None, None]
            psA = [None, None]
            pE0 = tc.alloc_tile_pool(name="psE0", bufs=1, space="PSUM")
            pE1 = tc.alloc_tile_pool(name="psE1", bufs=1, space="PSUM")
            pG0 = tc.alloc_tile_pool(name="psG0", bufs=1, space="PSUM")
            pG1 = tc.alloc_tile_pool(name="psG1", bufs=1, space="PSUM")
            pA0 = tc.alloc_tile_pool(name="psA0", bufs=1, space="PSUM")
            pA1 = tc.alloc_tile_pool(name="psA1", bufs=1, space="PSUM")
            psumE = (pE0, pE1)
            psumG = (pG0, pG1)
            psumA = (pA0, pA1)

            hbuf = [None, None]
            h_prev = [(hzero, None), (hzero, None)]  # (tile, ts) ts=None -> [128,KH,BG]
            c_prev = [czero, czero]
            state = {}

            def h_mov(g, kc):
                tile_, ts_ = h_prev[g]
                if ts_ is None:
                    return tile_[:, kc, :]
                return tile_[:, ts_, kc, :]

            e_pre = [None, None]  # pre-injected PS+PV psum tile per group
            phase_tok = [None, None]  # cross-group phase-offset tokens
            chunks = {}

            def get_chunk(t):
                k = t // Tc
                if k not in chunks:
                    gv = stream.tile([128, KM, Tc, BLOC], BF16, tag="gv", name="gv")
                    nc.sync.dma_start(
                        out=gv, in_=GV_d[:, :, k * Tc:(k + 1) * Tc, :].rearrange(
                            "m p t b -> p m t b"))
                    pv = stream.tile([128, KH, Tc, BLOC], BF16, tag="pv", name="pv")
                    nc.sync.dma_start(
                        out=pv, in_=PV_d[:, :, k * Tc:(k + 1) * Tc, :].rearrange(
                            "m p t b -> p m t b"))
                    chunks[k] = (gv, pv)
                    chunks.pop(k - 2, None)
                return chunks[k]

            def emit_preinject(g, t):
                """Off-chain: e_psum(t) <- ident@PS + ident@bcast(PV[t])."""
                ic = t % Tc
                pv = get_chunk(t)[1]
                e_ps = psumE[g].tile([128, KH, GL], F32, tag=f"e{g}", name=f"e{g}")
                for kc in range(KH):
                    nc.tensor.matmul(e_ps[:, kc, :], identb_sb,
                                     ps_bf[:, kc, g * GL:(g + 1) * GL],
                                     start=True, stop=False)
                    pvk = pv[:, kc, ic, g * BG:(g + 1) * BG]
                    bc = bass.AP(tensor=pvk.tensor, offset=pvk.offset,
                                 ap=[pvk.ap[0], [1, BG], [0, L]])
                    nc.tensor.matmul(e_ps[:, kc, :], identb_sb, bc,
                                     start=False, stop=False)
                e_pre[g] = e_ps

            def emit_P1(g, t):
                ic = t % Tc
                ts_ = t % HB
                gb = slice(g * BG, (g + 1) * BG)
                gv = get_chunk(t)[0]
                if ts_ == 0:
                    hbuf[g] = hout[g].tile([128, HB, KH, BG], BF16,
                                           tag=f"hb{g}", name=f"hb{g}")
                a = psumA[g].tile([128, 512], F32, tag=f"a{g}", name=f"a{g}")
                rv = a[:, 0:KH * BG].rearrange("p (m b) -> p m b", b=BG)
                e_ps = e_pre[g]
                rvb = wk[g].tile([128, KH, BG], BF16, tag=f"rvb{g}", name=f"rvb{g}")
                e_sb = wk[g].tile([128, KH, GL], BF16, tag=f"e{g}", name=f"esb{g}")
                # kc-half pipelined e-path: rv(m-half) -> rvb(half) ->
                # inject(half); one full etanh. The first rvb carries an
                # artificial bypass-dependency on the OTHER group's e tile to
                # enforce a phase offset between the two pipelined groups.
                for half in range(2):
                    ms = (0, 1) if half == 0 else (2, 3)
                    for m in ms:
                        for kc in range(KH):
                            nc.tensor.matmul(
                                rv[:, m, :], wr_sb[:, kc, m * 128:(m + 1) * 128],
                                h_mov(g, kc), start=(kc == 0), stop=(kc == KH - 1))
                    dst = rvb[:, ms[0]:ms[1] + 1].rearrange("p m b -> p (m b)")
                    src = a[:, ms[0] * BG:(ms[1] + 1) * BG]
                    tok = phase_tok[1 - g]
                    if half == 0 and tok is not None:
                        bc0 = bass.AP(tensor=tok.tensor, offset=tok.offset,
                                      ap=[tok.ap[0], [0, 2 * BG]])
                        nc.vector.scalar_tensor_tensor(
                            dst, src, 0.0, bc0, ALU.add, ALU.bypass)
                    else:
                        nc.vector.tensor_copy(dst, src)
                    for kc in ms:
                        rvk = rvb[:, kc, :]
                        bc = bass.AP(tensor=rvk.tensor, offset=rvk.offset,
                                     ap=[rvk.ap[0], [1, BG], [0, L]])
                        nc.tensor.matmul(e_ps[:, kc, :], identb_sb, bc,
                                         start=False, stop=True)
                nc.scalar.activation(
                    e_sb.rearrange("p k n -> p (k n)"),
                    e_ps.rearrange("p k n -> p (k n)"), AF.Tanh)
                phase_tok[g] = e_sb
                # PE: gates early: ident@GV + W_hh @ Hc
                gps = psumG[g].tile([128, KM, BG], F32, tag=f"g{g}", name=f"g{g}")
                for m in range(KM):
                    nc.tensor.matmul(gps[:, m, :], identb_sb,
                                     gv[:, m, ic, gb], start=True, stop=False)
                    for kc in range(KH):
                        nc.tensor.matmul(
                            gps[:, m, :], whh_sb[:, kc, m * 128:(m + 1) * 128],
                            h_mov(g, kc), start=False, stop=False)
                state[(g, t)] = (a, e_sb, gps, ts_)

            def emit_P2a(g, t):
                a, e_sb, gps, ts_ = state.pop((g, t))
                # PE: beta [48, BG] per-column accumulation
                bts = a[0:L, KH * BG:KH * BG + BG]
                for b in range(BG):
                    for kc in range(KH):
                        nc.tensor.matmul(
                            bts[:, b:b + 1],
                            e_sb[:, kc, b * L:(b + 1) * L],
                            wvec_sb[:, kc:kc + 1],
                            start=(kc == 0), stop=(kc == KH - 1))
                # Act: exp(beta + b_w)
                ex = wk[g].tile([L, BG], F32, tag=f"ex{g}", name=f"ex{g}")
                nc.scalar.activation(ex, bts, AF.Exp, bias=bw_sb[0:L, 0:1])
                # Pool: denominator; DVE: recip + alpha
                dsum = wk[g].tile([L, BG], F32, tag=f"ds{g}", name=f"ds{g}")
                nc.gpsimd.partition_all_reduce(dsum, ex, channels=L,
                                               reduce_op=bass_isa.ReduceOp.add)
                rd = wk[g].tile([L, BG], F32, tag=f"rd{g}", name=f"rd{g}")
                nc.vector.reciprocal(rd, dsum)
                al = wk[g].tile([L, BG], BF16, tag=f"al{g}", name=f"al{g}")
                nc.vector.tensor_tensor(al, ex, rd, ALU.mult)
                state[(g, t)] = (gps, al, ts_)

            def emit_P2b(g, t):
                gps, al, ts_ = state.pop((g, t))
                gb0 = g * BG
                # PE: Z @ alpha into the open gates PSUM group
                for m in range(KM):
                    for b in range(BG):
                        nc.tensor.matmul(
                            gps[:, m, b:b + 1],
                            z_sb[:, gb0 + b, m * 128:(m + 1) * 128],
                            al[:, b:b + 1], start=False, stop=True)
                # Act: gate tanh
                tg = wk[g].tile([128, KM, BG], F32, tag=f"tg{g}", name=f"tg{g}")
                nc.scalar.activation(
                    tg.rearrange("p m b -> p (m b)"),
                    gps.rearrange("p m b -> p (m b)"), AF.Tanh)
                ti = tg[:, 0:KH, :]
                tf = tg[:, KH:2 * KH, :]
                tgg = tg[:, 2 * KH:3 * KH, :]
                to = tg[:, 3 * KH:4 * KH, :]
                z1 = wk[g].tile([128, KH, BG], F32, tag=f"z1{g}", name=f"z1{g}")
                nc.vector.scalar_tensor_tensor(z1, tf, 1.0, c_prev[g], ALU.add, ALU.mult)
                z2 = wk[g].tile([128, KH, BG], F32, tag=f"z2{g}", name=f"z2{g}")
                nc.gpsimd.scalar_tensor_tensor(z2, ti, 1.0, tgg, ALU.add, ALU.mult)
                cn = st[g].tile([128, KH, BG], F32, tag=f"c{g}", name=f"c{g}")
                nc.vector.scalar_tensor_tensor(cn, z1, 0.5, z2, ALU.mult, ALU.add)
                tc_ = wk[g].tile([128, KH, BG], F32, tag=f"tc{g}", name=f"tc{g}")
                nc.scalar.activation(
                    tc_.rearrange("p k b -> p (k b)"),
                    cn.rearrange("p k b -> p (k b)"), AF.Tanh, scale=0.5)
                nc.vector.scalar_tensor_tensor(
                    hbuf[g][:, ts_], to, 1.0, tc_, ALU.add, ALU.mult)
                c_prev[g] = cn
                h_prev[g] = (hbuf[g], ts_)
                if ts_ == HB - 1 or t == T - 1:
                    nb = ts_ + 1
                    t0 = t - nb + 1
                    nc.sync.dma_start(
                        out=out_c[g, :, :, t0:t0 + nb, :],
                        in_=hbuf[g][:, :nb].rearrange("p t k b -> k p t b"))

            # cycle order tuned to steady-state ready times:
            # [P2b_B(t-1)] [P1_A(t)] [P2a_A(t) preinj_A(t+1)] [P1_B(t)]
            # [P2b_A(t)] [P2a_B(t) preinj_B(t+1)]
            emit_preinject(0, 0)
            emit_preinject(1, 0)
            for t in range(T):
                if t > 0:
                    emit_P2b(1, t - 1)
                emit_P1(0, t)
                emit_P2a(0, t)
                if t + 1 < T:
                    emit_preinject(0, t + 1)
                emit_P1(1, t)
                emit_P2b(0, t)
                emit_P2a(1, t)
                if t + 1 < T:
                    emit_preinject(1, t + 1)
            emit_P2b(1, T - 1)
            for p in (pA1, pA0, pG1, pG0, pE1, pE0):
                p.release()
    nc.finalize()
    return nc


# ---------------- host side ----------------

def prep_core_inputs(h_v, h_s, W, T=T_FULL):
    import ml_dtypes
    BF = ml_dtypes.bfloat16
    srow = np.concatenate([
        np.full(H, 0.5, np.float32), np.full(H, 0.5, np.float32),
        np.ones(H, np.float32), np.full(H, 0.5, np.float32)])
    W_ih = W["W_ih"] * srow[:, None]
    W_hh = W["W_hh"] * srow[:, None] * 0.5
    W_R = W["W_R"] * 0.5
    WS_T = np.ascontiguousarray(W["W_S"].T).astype(BF)
    WV_T = np.ascontiguousarray(W["W_V"].T).astype(BF)
    WihV_T = np.ascontiguousarray(W_ih[:, :DV].T).astype(BF)
    WihS_T = np.ascontiguousarray(W_ih[:, DV:].T).astype(BF)
    Whh_T = np.ascontiguousarray(W_hh.T).astype(BF)
    WR_T = np.ascontiguousarray(W_R.T).astype(BF)
    wvec = np.ascontiguousarray(W["W_w"][0].reshape(KH, 128).T).astype(BF)
    biasRSV = np.ascontiguousarray(
        (W["b_S"] + W["b_R"] + W["b_V"]).reshape(KH, 128).T)
    biasIH = np.ascontiguousarray(
        ((W["b_ih"] + W["b_hh"]) * srow).reshape(KM, 128).T)
    bw128 = np.full((128, 1), W["b_w"][0], np.float32)
    identb = np.eye(128, dtype=np.float32).astype(BF)
    maps = []
    for c in range(NCORES):
        bs = slice(c * BLOC, (c + 1) * BLOC)
        hvT = np.ascontiguousarray(
            h_v[bs, :T].transpose(2, 1, 0).reshape(DV, T * BLOC)).astype(BF)
        hsT = np.ascontiguousarray(
            h_s[bs].transpose(2, 0, 1).reshape(DS, BLOC * L)).astype(BF)
        maps.append({
            "hvT": hvT, "hsT": hsT, "WS_T": WS_T, "WV_T": WV_T,
            "WihV_T": WihV_T, "WihS_T": WihS_T, "Whh_T": Whh_T, "WR_T": WR_T,
            "wvec": wvec, "biasRSV": biasRSV, "biasIH": biasIH, "bw128": bw128,
            "identb": identb,
        })
    return maps


_NC_CACHE = {}


def kernel(**inputs):
    h_v = np.asarray(inputs["h_v"], dtype=np.float32)
    h_s = np.asarray(inputs["h_s"], dtype=np.float32)
    W = {k: np.asarray(v, dtype=np.float32) for k, v in inputs.items()}
    if "full" not in _NC_CACHE:
        _NC_CACHE["full"] = build_nc(T=T_FULL)
    nc = _NC_CACHE["full"]
    maps = prep_core_inputs(h_v, h_s, W, T=T_FULL)
    res = run_bass_kernel_spmd(nc, maps, list(range(NCORES)))
    outs = []
    for c in range(NCORES):
        arr = res.results[c]["out_c"]  # [NG, KH, 128, T, BG] bf16 (Hc = 2h)
        # -> [BLOC, T, H]: b_local = g*BG + b, H = k*128 + p
        full = np.transpose(np.asarray(arr, dtype=np.float32),
                            (0, 4, 3, 1, 2)).reshape(BLOC, T_FULL, H)
        outs.append(full * 0.5)
    return np.concatenate(outs, axis=0).astype(np.float32)


if __name__ == "__main__":
    nc = build_nc(T=32, Tc=32)
    print("built ok")
